# revision 17
# baseline (speedup 1.0000x reference)
"""AdaLoRA routed-LoRA kernel for 8 Trainium2 NeuronCores (v5-final).

Problem (nn_AdaLoRA): per token t with expert index i:
    ds[t, :]  = slots[t, :] @ down_table[i]            # [1024] @ [1024, 16]
    out[t, :] = (ds[t, :] @ up_table[i]) / sqrt(16)    # [16] @ [16, 1024]

Sharding: data-parallel over batch (B=8 -> one batch row per core; LoRA
tables replicated). Per core: 256 tokens = 2 tiles of 128 tokens.

History: 91.8 -> 69.0 (v3, int8 tables + in-flight f16 cast) -> 64.3
(v4, per-c down gathers + gather-buffer ring fix + interleaved gather
order) -> 64.2 (v5, FWL-friendly full-width lhsT + materialized
replicate rhs + direct-f16 accumulator writeout). Later experiments
(v6-v9: finer gather splits, emission reorders, fused transpose+
replicate MM) measured 65.5-67.3 on HW - the cross-engine queue
couplings moved rather than shrank - so v5 is kept as final.

Design notes (from NTFF profiles):
- Both tables int8 (per-expert absmax), SWDGE indirect-gather casts
  int8->f16 in flight. fp8e4m3 for the up table would halve its write
  bytes but fails numerically: host-sim rel err 3.4e-2 vs the 2e-2
  gate (int8 sim 1.128e-2 matches HW 1.131e-2). DVE needs f16 operands
  for 2x mode (8-bit operands drop it to 1x), so raw-int8-in-SBUF
  loses more on compute than it saves on fabric.
- The gather stream runs at 420-434 GB/s write-side (435 = fabric
  ceiling); writes, not HBM reads, bound the window. Gather-op
  completions are byte-paced (verified +0.2-0.9us vs cumulative-byte
  model; no SDMA-engine skew worth modeling).
- Down gathers are one op per rank-group c (4KB descriptors) so the
  down-projection pipelines per chunk; gather-buffer rings are sized
  so SWDGE descriptor generation never stalls (a 2-deep ring cost v3
  a ~4us mid-window bubble).
- Down-proj per (t, c): ACT-path ranks (DVE TT multiply at 2x + ACT
  activation accum) and DVE-STT ranks (fused mult+accum at 1x),
  balanced so both engines run ~33us. Accumulators write f16 ds16
  columns directly (allow_low_precision; the engine accumulator is
  f32 internally).
- lhsT build per (t, c): PE transpose [128,4]->[4,128], DVE copy to a
  materialized broadcast [4,512] (a stride-0 rhs AP on the PE moving
  operand costs ~1.25us vs ~0.45us materialized), replicate-MM with
  full-128-col lhsT (FWL on; narrower stationaries run MMs ~1.6x
  slower), DVE mask. Up-MMs: 64 of [128,128]@[128,512] f16 at ~379ns.
- MM emission chases the gather arrivals; finishers run one chunk
  behind the rank ops so DVE never stalls on ACT's accumulators.
- Epilogue note: the framework resets all 254 semaphores at exit
  (~7us across engines) regardless of kernel structure - fixed cost.

HW findings (CoreSim/TimelineSim disagree!):
- multi-offset indirect DMA (offset AP [128,k>1]) returns garbage on
  real HW although CoreSim models it fine -> single-offset only.
- tensor_tensor_reduce faults the device -> use scalar_tensor_tensor.
- gpsimd tensor ops cannot read PSUM (BIR verifier).
- matmul stationary APs must have exactly one free dimension (no
  stride-0 broadcast lhsT).
- Emission order IS the dependency order: an MM emitted before the op
  that writes its lhsT reads garbage (deps only look backward).
"""

import numpy as np

B, K, DIM, RANK, NE = 8, 256, 1024, 16, 4096
ROW = DIM * RANK  # 16384 int8 elements per down-table row
SCALE = 1.0 / 4.0  # 1/sqrt(RANK)
P = 128
N_TILE = K // P  # 2 token tiles per core
RSLOT = 4  # ranks per partition in the up gather
TPG = P // RSLOT  # 32 tokens per up group
NGRP = P // TPG  # 4 up groups per tile
N_CORES = 8
CBLOB = P + NGRP * P + P  # ident | m4g | e4pad columns

_CACHE = {}


def _build():
    from concourse import bacc, bass, mybir, tile

    f32 = mybir.dt.float32
    f16 = mybir.dt.float16
    i8 = mybir.dt.int8
    i32 = mybir.dt.int32
    mult = mybir.AluOpType.mult
    Copy = mybir.ActivationFunctionType.Copy

    nc = bacc.Bacc("TRN2", target_bir_lowering=False, dynamic_dma_scratch_size=65536)
    # idxcat[:, 0:2] = down row idx per (p, t); [:, 2:10] = up4 row idx per (p, t*4+g)
    idxcat = nc.declare_dram_parameter("idxcat", [P, 2 + N_TILE * NGRP], i32, isOutput=False)
    slots = nc.declare_dram_parameter("slots", [K, DIM], f16, isOutput=False)
    cs2 = nc.declare_dram_parameter("cs2", [P, N_TILE], f32, isOutput=False)
    down = nc.declare_dram_parameter("down", [NE, ROW], i8, isOutput=False)
    up4 = nc.declare_dram_parameter("up4", [NE * RSLOT, RSLOT * DIM], i8, isOutput=False)
    cblob = nc.declare_dram_parameter("cblob", [P, CBLOB], f16, isOutput=False)
    out = nc.declare_dram_parameter("out", [K, DIM], f16, isOutput=True)

    with tile.TileContext(nc) as tc:
        with (
            tc.tile_pool(name="io", bufs=2) as io_pool,
            tc.tile_pool(name="gath", bufs=6) as gpool,
            tc.tile_pool(name="upg", bufs=8) as upool,
            tc.tile_pool(name="prod", bufs=3) as ppool,
            tc.tile_pool(name="misc", bufs=1) as mpool,
            tc.tile_pool(name="ds", bufs=3) as dspool,
            tc.tile_pool(name="psT", bufs=2, space="PSUM") as psT,
            tc.tile_pool(name="psR", bufs=2, space="PSUM") as psR,
            tc.tile_pool(name="psO", bufs=2, space="PSUM") as psO,
        ):
            # ---- index load first: it gates every gather ----
            idx_sb = mpool.tile([P, 2 + N_TILE * NGRP], i32)
            nc.sync.dma_start(out=idx_sb[:], in_=idxcat[:, :])

            # ---- remaining loads on the second HWDGE ring (ACT) ----
            slots_all = mpool.tile([P, N_TILE, DIM], f16)
            nc.scalar.dma_start(
                out=slots_all[:], in_=slots[:, :].rearrange("(t p) d -> p t d", p=P)
            )
            cb = mpool.tile([P, CBLOB], f16)
            nc.scalar.dma_start(out=cb[:], in_=cblob[:, :])
            ident = cb[:, 0:P]
            m4g = cb[:, P : P + NGRP * P]
            e4_sb = cb[0:RSLOT, P + NGRP * P : P + NGRP * P + P]
            cs_sb = mpool.tile([P, N_TILE], f32)
            nc.scalar.dma_start(out=cs_sb[:], in_=cs2[:, :])

            # ---- indirect gathers: all issued up front on the SWDGE queue ----
            dch = {}
            upc = {}

            def emit_down(t, c):
                d = gpool.tile([P, RSLOT, DIM], f16, tag="dch")
                nc.gpsimd.indirect_dma_start(
                    out=d[:].rearrange("p r d -> p (r d)"),
                    out_offset=None,
                    in_=down[:],
                    in_offset=bass.IndirectOffsetOnAxis(ap=idx_sb[:, t : t + 1], axis=0),
                    element_offset=c * RSLOT * DIM,
                )
                dch[t, c] = d

            def emit_up(t, g):
                u = upool.tile([P, RSLOT * DIM], f16, tag="upc")
                nc.gpsimd.indirect_dma_start(
                    out=u[:],
                    out_offset=None,
                    in_=up4[:],
                    in_offset=bass.IndirectOffsetOnAxis(
                        ap=idx_sb[:, 2 + t * NGRP + g : 3 + t * NGRP + g], axis=0
                    ),
                )
                upc[t, g] = u

            for c in range(RSLOT):
                emit_down(0, c)
            emit_down(1, 0)
            emit_down(1, 1)
            emit_up(0, 0)
            emit_up(0, 1)
            emit_up(0, 2)
            emit_down(1, 2)
            emit_up(0, 3)
            emit_down(1, 3)
            for g in range(NGRP):
                emit_up(1, g)

            lhsT_all = mpool.tile([P, N_TILE, RSLOT, NGRP, P], f16)
            scr_act = mpool.tile([P, DIM], f16)
            scr_dve = mpool.tile([P, DIM], f16)

            # ---- down projection per (t, c): 4 ranks {4rp+c} ----
            N_ACT = {  # rank-slots on the ACT path per (t, c)
                (0, 0): 3, (0, 1): 3, (0, 2): 3, (0, 3): 2,
                (1, 0): 3, (1, 1): 3, (1, 2): 3, (1, 3): 1,
            }

            def emit_ranks(t, c):
                d = dch[t, c]
                na = N_ACT[t, c]
                ds16 = dspool.tile([P, RSLOT], f16, tag="ds16")
                with nc.allow_low_precision(reason="accumulator is f32; f16 on writeout"):
                    for rp in range(na):  # ACT path first: feed ACT asap
                        prod = ppool.tile([P, DIM], f16, tag="prod")
                        nc.vector.tensor_tensor(
                            out=prod[:], in0=slots_all[:, t, :], in1=d[:, rp, :], op=mult
                        )
                        nc.scalar.activation(
                            out=scr_act[:],
                            in_=prod[:],
                            func=Copy,
                            accum_out=ds16[:, rp : rp + 1],
                        )
                    for rp in range(na, RSLOT):
                        nc.vector.scalar_tensor_tensor(
                            out=scr_dve[:],
                            in0=slots_all[:, t, :],
                            scalar=1.0,
                            in1=d[:, rp, :],
                            op0=mult,
                            op1=mult,
                            accum_out=ds16[:, rp : rp + 1],
                        )
                return ds16

            def emit_fin(t, c, ds16):
                # ds16 [tok, rp] -> dsT [rp, tok] -> dsT4 [rp, (g,tok)] ->
                # rep[p, (g,m)] = dsT[p%4, m] -> mask [p//4 == m%32, g-match]
                dsT_psum = psT.tile([RSLOT, P], f16, space="PSUM", tag="dsT")
                nc.tensor.transpose(out=dsT_psum[:], in_=ds16[:], identity=ident)
                dsT4 = dspool.tile([RSLOT, NGRP, P], f16, tag="dsT4")
                nc.vector.tensor_copy(
                    out=dsT4[:],
                    in_=dsT_psum[:]
                    .rearrange("q (one c) -> q one c", one=1)
                    .broadcast_to((RSLOT, NGRP, P)),
                )
                rep = psR.tile([P, NGRP * P], f32, space="PSUM", tag="rep")
                nc.tensor.matmul(
                    out=rep[:],
                    lhsT=e4_sb,
                    rhs=dsT4[:].rearrange("q g c -> q (g c)"),
                    start=True,
                    stop=True,
                )
                nc.vector.tensor_tensor(
                    out=lhsT_all[:, t, c, :, :].rearrange("p g c -> p (g c)"),
                    in0=rep[:],
                    in1=m4g,
                    op=mult,
                )

            out_psum = {}
            n_mm = {}
            for t in range(N_TILE):
                op_t = psO.tile([P, DIM], f32, space="PSUM", tag="outp")
                out_psum[t] = op_t
                n_mm[t, 0] = 0
                n_mm[t, 1] = 0

            def emit_mm(t, g, c):
                for n in range(2):
                    n0, n1 = n * 512, (n + 1) * 512
                    n_mm[t, n] += 1
                    nc.tensor.matmul(
                        out=out_psum[t][:, n0:n1],
                        lhsT=lhsT_all[:, t, c, g, :],
                        rhs=upc[t, g][:, c * DIM + n0 : c * DIM + n1],
                        start=(n_mm[t, n] == 1),
                        stop=(n_mm[t, n] == NGRP * RSLOT),
                    )

            def emit_out(t, dve_half=False):
                out_sb = io_pool.tile([P, DIM], f16, tag="osb")
                for h in range(2):
                    h0, h1 = h * 512, (h + 1) * 512
                    if dve_half and h == 1:
                        nc.vector.tensor_scalar(
                            out=out_sb[:, h0:h1],
                            in0=out_psum[t][:, h0:h1],
                            scalar1=cs_sb[:, t : t + 1],
                            scalar2=None,
                            op0=mult,
                        )
                    else:
                        nc.scalar.activation(
                            out=out_sb[:, h0:h1],
                            in_=out_psum[t][:, h0:h1],
                            func=Copy,
                            scale=cs_sb[:, t : t + 1],
                        )
                    nc.sync.dma_start(
                        out=out[t * P : (t + 1) * P, h0:h1], in_=out_sb[:, h0:h1]
                    )

            # ---- emission in expected-arrival order ----
            ds = {}
            ds[0, 0] = emit_ranks(0, 0)
            ds[0, 1] = emit_ranks(0, 1)
            emit_fin(0, 0, ds[0, 0])
            ds[0, 2] = emit_ranks(0, 2)
            emit_fin(0, 1, ds[0, 1])
            ds[0, 3] = emit_ranks(0, 3)
            emit_fin(0, 2, ds[0, 2])
            ds[1, 0] = emit_ranks(1, 0)
            emit_fin(0, 3, ds[0, 3])

            # PE: t0 MMs chase U(t0,g) arrivals; c3 joins once its lhsT lands
            emit_mm(0, 0, 0)
            emit_mm(0, 0, 1)
            emit_mm(0, 1, 0)
            emit_mm(0, 1, 1)
            emit_mm(0, 0, 2)
            emit_mm(0, 1, 2)
            emit_mm(0, 0, 3)
            emit_mm(0, 1, 3)
            ds[1, 1] = emit_ranks(1, 1)
            emit_fin(1, 0, ds[1, 0])
            for c in range(RSLOT):
                emit_mm(0, 2, c)
            ds[1, 2] = emit_ranks(1, 2)
            emit_fin(1, 1, ds[1, 1])
            for c in range(RSLOT):
                emit_mm(0, 3, c)
            ds[1, 3] = emit_ranks(1, 3)
            emit_fin(1, 2, ds[1, 2])
            emit_out(0)

            # PE: t1 MMs; c3 (late lhsT) joins after every group's c0..c2
            emit_mm(1, 0, 0)
            emit_mm(1, 0, 1)
            emit_mm(1, 0, 2)
            emit_mm(1, 1, 0)
            emit_mm(1, 1, 1)
            emit_mm(1, 1, 2)
            emit_mm(1, 2, 0)
            emit_mm(1, 2, 1)
            emit_mm(1, 2, 2)
            emit_fin(1, 3, ds[1, 3])
            emit_mm(1, 3, 0)
            emit_mm(1, 3, 1)
            emit_mm(1, 3, 2)
            emit_mm(1, 0, 3)
            emit_mm(1, 1, 3)
            emit_mm(1, 2, 3)
            emit_mm(1, 3, 3)
            emit_out(1, dve_half=True)
    nc.compile()
    return nc


def _get_nc():
    if "nc" not in _CACHE:
        _CACHE["nc"] = _build()
    return _CACHE["nc"]


def _prep_in_maps(slots, indices, down_proj_values, up_proj_values):
    slots = np.ascontiguousarray(np.asarray(slots, dtype=np.float32).astype(np.float16))
    indices = np.ascontiguousarray(np.asarray(indices).astype(np.int32))
    downT = np.asarray(down_proj_values, dtype=np.float32).transpose(0, 2, 1)  # [NE,R,D]
    up = np.asarray(up_proj_values, dtype=np.float32)  # [NE,R,D]

    # per-expert int8 quantization
    s_d = np.abs(downT).max(axis=(1, 2)) / 127.0  # [NE]
    s_u = np.abs(up).max(axis=(1, 2)) / 127.0
    # rank order (c, rp): rank r = 4*rp + c at block c*4096 + rp*1024
    perm = np.array([4 * rp + c for c in range(RSLOT) for rp in range(RSLOT)])
    down_q = np.ascontiguousarray(
        np.clip(np.round(downT[:, perm, :] / s_d[:, None, None]), -127, 127)
        .astype(np.int8)
        .reshape(NE, ROW)
    )
    up_q = np.ascontiguousarray(
        np.clip(np.round(up / s_u[:, None, None]), -127, 127)
        .astype(np.int8)
        .reshape(NE * RSLOT, RSLOT * DIM)
    )

    # host constants: cblob = ident [P,P] | m4g [P, NGRP*P] | e4 (padded) [P,P]
    ident_c = np.eye(P, dtype=np.float16)
    p_i = np.arange(P)[:, None, None]
    g_i = np.arange(NGRP)[None, :, None]
    col = np.arange(P)[None, None, :]
    m4g_c = (
        ((p_i // RSLOT) == (col % TPG)) & ((col // TPG) == g_i)
    ).astype(np.float16).reshape(P, NGRP * P)
    e4_pad = np.zeros((P, P), np.float16)
    e4_pad[:RSLOT, :] = (
        np.arange(RSLOT)[:, None] == (np.arange(P)[None, :] % RSLOT)
    ).astype(np.float16)
    cblob = np.ascontiguousarray(np.concatenate([ident_c, m4g_c, e4_pad], axis=1))

    p = np.arange(P)
    j, rp = p // RSLOT, p % RSLOT
    t_i = np.arange(N_TILE)[:, None, None]
    g_i2 = np.arange(NGRP)[None, :, None]
    toks = P * t_i + TPG * g_i2 + j[None, None, :]  # [N_TILE, NGRP, P]

    in_maps = []
    for i in range(N_CORES):
        idx_i = indices[i]  # [K]
        idxcat = np.empty((P, 2 + N_TILE * NGRP), np.int32)
        for t in range(N_TILE):
            idxcat[:, t] = idx_i[t * P : (t + 1) * P]
        up_rows = idx_i[toks] * RSLOT + rp[None, None, :]  # [N_TILE, NGRP, P]
        for t in range(N_TILE):
            for g in range(NGRP):
                idxcat[:, 2 + t * NGRP + g] = up_rows[t, g]
        cs_tok = (s_d[idx_i] * s_u[idx_i] * SCALE).astype(np.float32)  # [K]
        cs2 = np.stack([cs_tok[t * P : (t + 1) * P] for t in range(N_TILE)], axis=1)
        in_maps.append(
            {
                "idxcat": np.ascontiguousarray(idxcat),
                "slots": slots[i],
                "cs2": np.ascontiguousarray(cs2),
                "down": down_q,
                "up4": up_q,
                "cblob": cblob,
            }
        )
    return in_maps


def _run(in_maps, trace=False):
    from concourse.bass_utils import run_bass_kernel_spmd

    nc = _get_nc()
    return run_bass_kernel_spmd(
        nc, in_maps, core_ids=list(range(N_CORES)), trace=trace
    )


def kernel(slots, indices, down_proj_values, up_proj_values):
    in_maps = _prep_in_maps(slots, indices, down_proj_values, up_proj_values)
    res = _run(in_maps)
    out = np.stack([res.results[i]["out"] for i in range(N_CORES)], axis=0)
    return out.astype(np.float32)
